# revision 2
# baseline (speedup 1.0000x reference)
"""Trainium2 Bass kernel for nn_Batch_Edge (gnn_message_passing).

Computation (see reference):
    node_embed = last_node_batch @ W_embed + b_embed          # [B, H]
    stack      = concat([h, node_embed[seg]], axis=1)         # [N, 2H]
    out        = tanh(stack @ W1 + b1); out = tanh(out @ W2 + b2)
    edges      = out @ W3 + b3                                # [N, 2]
    result     = edges reshaped to [B, max_nodes*2]  (no padding: all graphs full)

Strategy: shard 512 graphs (131072 nodes) contiguously across 8 cores (64
graphs / 16384 nodes each).  Activations are feature-on-partition
([feature, node]); matmuls run in bf16 (1 col/cycle on the PE — fp32/f32r
run in the 4x-slower fp32 HIGH mode on TRN2 hardware).  The per-graph
embedding contribution C = node_embed @ W1[H:, :] + b1 is computed on the
host (tiny: B x 2H) and added into the L1 PSUM accumulation by a K=2
indicator matmul, so no vector-engine broadcast adds are needed.  Per
1024-node chunk: one 2048-col tanh per MLP layer (L1 reads PSUM directly;
L2 goes PSUM -> DVE b2-add -> SBUF bf16 -> tanh), and the 2-row L3 output
accumulates into partitions 0-1 of the retiring L2 PSUM tile, which keeps
the whole pipeline inside the 8 PSUM banks with double buffering.
"""

import os
import numpy as np

B = 512
NPG = 256               # nodes per graph
N = B * NPG             # 131072
HID = 128
NCORES = 8
GPC = B // NCORES       # 64 graphs per core
NPC = N // NCORES       # 16384 nodes per core
PAD_VALUE = -10000.0

CH = 1024               # nodes handled per main-loop chunk
NCH = NPC // CH         # 16 chunks per core

LAST_RESULT = None      # BassKernelResults of the most recent device run
_CACHE = {}


def _numpy_ref(last_node_batch, h, W_embed, b_embed, W1, b1, W2, b2, W3, b3,
               segment_ids, max_nodes):
    """Exact host fallback (used only if inputs don't match the expected
    uniform-graph structure)."""
    lnb = np.asarray(last_node_batch, np.float32)
    h = np.asarray(h, np.float32)
    seg = np.asarray(segment_ids).astype(np.int64)
    b = lnb.shape[0]
    n = h.shape[0]
    mn = int(np.asarray(max_nodes))
    node_embed = lnb @ np.asarray(W_embed, np.float32) + np.asarray(b_embed, np.float32)
    stack = np.concatenate([h, node_embed[seg]], axis=1)
    out = np.tanh(stack @ np.asarray(W1, np.float32) + np.asarray(b1, np.float32))
    out = np.tanh(out @ np.asarray(W2, np.float32) + np.asarray(b2, np.float32))
    edges = out @ np.asarray(W3, np.float32) + np.asarray(b3, np.float32)
    counts = np.zeros(b, np.int64)
    np.add.at(counts, seg, 1)
    offsets = np.cumsum(counts) - counts
    pos = np.arange(n) - offsets[seg]
    padded = np.full((b, mn, 2), PAD_VALUE, np.float32)
    padded[seg, pos] = edges
    return padded.reshape(b, mn * 2)


def _build():
    """Build + compile the per-core Bass program (identical on all cores)."""
    import concourse.bacc as bacc
    import concourse.mybir as mybir
    import concourse.tile as tile

    f32 = mybir.dt.float32
    bf16 = mybir.dt.bfloat16
    Tanh = mybir.ActivationFunctionType.Tanh

    nc = bacc.Bacc("TRN2", target_bir_lowering=False, debug=False, enable_asserts=False)

    # wpk columns (bf16): W1h[0:256] W2a[256:512] W2b[512:768] W3a[768:770]
    #                     W3b[770:772] E[772:1284] (E lives on partitions 0-1)
    hT = nc.dram_tensor("hT", [128, NPC], bf16, kind="ExternalInput").ap()
    wpk = nc.dram_tensor("wpk", [128, 1284], bf16, kind="ExternalInput").ap()
    # per-graph L1 contribution, pair-packed: ctp[i, p*256+f] = C[2p+i, f]
    ctp = nc.dram_tensor("ctp", [2, GPC // 2 * 256], bf16, kind="ExternalInput").ap()
    b2s = nc.dram_tensor("b2s", [128, 2], f32, kind="ExternalInput").ap()
    out_d = nc.dram_tensor("out", [2, NPC], f32, kind="ExternalOutput").ap()

    with tile.TileContext(nc) as tc:
        with (
            tc.tile_pool(name="w", bufs=1) as wp,
            tc.tile_pool(name="io", bufs=3) as io,
            tc.tile_pool(name="act", bufs=2) as ac,
            tc.tile_pool(name="ps", bufs=2, space="PSUM") as ps,
        ):
            s_w = wp.tile([128, 1284], bf16, tag="wpk")
            nc.sync.dma_start(out=s_w[:], in_=wpk)
            s_ct = wp.tile([2, GPC // 2 * 256], bf16, tag="ctp")
            nc.sync.dma_start(out=s_ct[:], in_=ctp)
            s_b2 = wp.tile([128, 2], f32, tag="b2s")
            nc.sync.dma_start(out=s_b2[:], in_=b2s)

            h_tiles = {}
            t_h0 = io.tile([128, CH], bf16, tag="h")
            nc.sync.dma_start(out=t_h0[:, 0:CH // 2], in_=hT[:, 0:CH // 2])
            nc.sync.dma_start(out=t_h0[:, CH // 2:CH], in_=hT[:, CH // 2:CH])
            h_tiles[0] = t_h0

            s_w1 = s_w[:, 0:256]
            s_w2a = s_w[:, 256:512]
            s_w2b = s_w[:, 512:768]
            s_w3a = s_w[:, 768:770]
            s_w3b = s_w[:, 770:772]
            s_E = s_w[0:2, 772:1284]

            for c in range(NCH):
                if c in h_tiles:
                    t_h = h_tiles.pop(c)
                else:
                    t_h = io.tile([128, CH], bf16, tag="h")
                    nc.sync.dma_start(out=t_h[:], in_=hT[:, c * CH:(c + 1) * CH])

                # L1: z1[:, m*1024+s*512 ...] = W1h_m.T @ h + C (indicator mm)
                z1 = ps.tile([128, 2048], f32, tag="z")
                for m in (0, 1):
                    for s in (0, 1):
                        o = m * 1024 + s * 512
                        pair = c * 2 + s
                        nc.tensor.matmul(
                            z1[:, o:o + 512],
                            s_w1[:, m * 128:m * 128 + 128],
                            t_h[:, s * 512:s * 512 + 512],
                            start=True, stop=False,
                        )
                        nc.tensor.matmul(
                            z1[:, o:o + 512],
                            s_ct[:, pair * 256 + m * 128:pair * 256 + m * 128 + 128],
                            s_E[:, 0:512],
                            start=False, stop=True,
                        )
                y1 = ac.tile([128, 2048], bf16, tag="y1")
                nc.scalar.activation(y1[:], z1[:], Tanh)

                # L2: z2[:, m*1024 + ...] = W2_m.T @ y1 (both k-halves)
                z2 = ps.tile([128, 2048], f32, tag="z")
                for m in (0, 1):
                    for s in (0, 1):
                        o = m * 1024 + s * 512
                        nc.tensor.matmul(
                            z2[:, o:o + 512],
                            s_w2a[:, m * 128:m * 128 + 128],
                            y1[:, s * 512:s * 512 + 512],
                            start=True, stop=False,
                        )
                        nc.tensor.matmul(
                            z2[:, o:o + 512],
                            s_w2b[:, m * 128:m * 128 + 128],
                            y1[:, 1024 + s * 512:1024 + s * 512 + 512],
                            start=False, stop=True,
                        )
                y2s = ac.tile([128, 2048], bf16, tag="y2s")
                nc.vector.tensor_scalar_add(y2s[:, 0:1024], z2[:, 0:1024], s_b2[:, 0:1])
                nc.vector.tensor_scalar_add(y2s[:, 1024:2048], z2[:, 1024:2048], s_b2[:, 1:2])
                y2 = ac.tile([128, 2048], bf16, tag="y2")
                nc.scalar.activation(y2[:], y2s[:], Tanh)

                # L3: edges^T accumulate into partitions 0-1 of the retiring
                # z2 tile (banks 0-1), then one DVE evict to SBUF.
                for s in (0, 1):
                    o = s * 512
                    nc.tensor.matmul(
                        z2[0:2, o:o + 512], s_w3a,
                        y2[:, o:o + 512],
                        start=True, stop=False,
                    )
                    nc.tensor.matmul(
                        z2[0:2, o:o + 512], s_w3b,
                        y2[:, 1024 + o:1024 + o + 512],
                        start=False, stop=True,
                    )
                ed = io.tile([2, CH], f32, tag="ed")
                nc.vector.tensor_copy(ed[:], z2[0:2, 0:1024])
                nc.sync.dma_start(out=out_d[:, c * CH:(c + 1) * CH], in_=ed[:])

    nc.compile()
    return nc


def _to_bf16(x):
    import ml_dtypes
    return np.ascontiguousarray(x.astype(ml_dtypes.bfloat16))


def kernel(last_node_batch, h, W_embed, b_embed, W1, b1, W2, b2, W3, b3,
           segment_ids, max_nodes):
    global LAST_RESULT
    lnb = np.asarray(last_node_batch, np.float32)
    h = np.asarray(h, np.float32)
    seg = np.asarray(segment_ids)
    mn = int(np.asarray(max_nodes))

    expected_seg = np.repeat(np.arange(B, dtype=seg.dtype), NPG)
    if not (lnb.shape == (B, HID) and h.shape == (N, HID) and mn == NPG
            and seg.shape == (N,) and np.array_equal(seg, expected_seg)):
        return _numpy_ref(last_node_batch, h, W_embed, b_embed, W1, b1, W2, b2,
                          W3, b3, segment_ids, max_nodes)

    import sys
    try:
        import antenv.axon_hooks  # noqa: F401
    except ImportError:
        # bass_utils imports this unconditionally when tracing is requested
        # (e.g. BASS_TRACE set in the environment); provide a no-op fallback
        # so tracing degrades instead of crashing.
        import types
        _m = types.ModuleType("antenv.axon_hooks")
        _m.get_axon_ntff_profile_hook = lambda: None
        _m.set_axon_ntff_profile_hook = lambda h: None
        sys.modules["antenv.axon_hooks"] = _m

    from concourse.bass_utils import run_bass_kernel_spmd

    if "nc" not in _CACHE:
        _CACHE["nc"] = _build()
    nc = _CACHE["nc"]

    W1 = np.asarray(W1, np.float32)
    W2 = np.asarray(W2, np.float32)
    W3 = np.asarray(W3, np.float32)
    b1v = np.asarray(b1, np.float32)
    b2v = np.asarray(b2, np.float32)
    b3v = np.asarray(b3, np.float32)

    # per-graph L1 contribution: C = (lnb @ W_embed + b_embed) @ W1[H:] + b1
    ne = lnb @ np.asarray(W_embed, np.float32) + np.asarray(b_embed, np.float32)
    C = ne @ W1[HID:, :] + b1v                                  # [B, 2H]

    E = np.zeros((128, 512), np.float32)
    E[0, 0:256] = 1.0
    E[1, 256:512] = 1.0
    wpk = _to_bf16(np.concatenate(
        [W1[:HID, :], W2[:HID, :], W2[HID:, :], W3[:HID, :], W3[HID:, :], E],
        axis=1))
    b2s = np.ascontiguousarray(np.stack([b2v[:HID], b2v[HID:]], axis=1))

    in_maps = []
    for c in range(NCORES):
        Cc = C[c * GPC:(c + 1) * GPC]                           # [64, 256]
        ctp = Cc.reshape(GPC // 2, 2, 256).transpose(1, 0, 2).reshape(2, -1)
        m = {
            "wpk": wpk,
            "ctp": _to_bf16(ctp),
            "b2s": b2s,
            "hT": _to_bf16(np.ascontiguousarray(h[c * NPC:(c + 1) * NPC].T)),
        }
        in_maps.append(m)

    trace = bool(int(os.environ.get("KERNEL_TRACE", "0")))
    res = run_bass_kernel_spmd(nc, in_maps, core_ids=list(range(NCORES)),
                               trace=trace)
    LAST_RESULT = res

    out = np.empty((B, NPG * 2), np.float32)
    for c in range(NCORES):
        od = res.results[c]["out"]          # [2, NPC]; [cc, n] = edges[n, cc]
        blk = od.reshape(2, GPC, NPG).transpose(1, 2, 0).reshape(GPC, NPG * 2)
        out[c * GPC:(c + 1) * GPC] = blk
    out += np.tile(b3v, NPG)[None, :]
    return out


# revision 7
# speedup vs baseline: 1.4741x; 1.4741x over previous
"""Trainium2 Bass kernel for nn_Batch_Edge (gnn_message_passing).

Computation (see reference):
    node_embed = last_node_batch @ W_embed + b_embed          # [B, H]
    stack      = concat([h, node_embed[seg]], axis=1)         # [N, 2H]
    out        = tanh(stack @ W1 + b1); out = tanh(out @ W2 + b2)
    edges      = out @ W3 + b3                                # [N, 2]
    result     = edges reshaped to [B, max_nodes*2]  (no padding: all graphs full)

Strategy: shard 512 graphs (131072 nodes) contiguously across 8 cores (64
graphs / 16384 nodes each).  Activations are feature-on-partition
([feature, node]); matmuls run in bf16 (1 col/cycle on the PE — fp32/f32r
run in the 4x-slower fp32 HIGH mode on TRN2 hardware).  The per-graph
embedding contribution C = node_embed @ W1[H:, :] + b1 is computed on the
host (tiny: B x 2H) and added into the L1 PSUM accumulation by K=1
rank-1 matmuls (C column x ones row), so no vector-engine broadcast adds
are needed.  The loop over 768-node chunks is software-pipelined three
stages deep — iteration i issues L1 matmuls for chunk i, L2 for chunk
i-1, and L3 for chunk i-2 — so every tensor-engine instruction depends
only on results from previous iterations and the PE never stalls (which
would re-throttle the HAM clock gate).  PSUM: z1 + z2 are 3-bank tiles,
L3 accumulates in a dedicated 2-tile pool, 8 banks total.
"""

import os
import numpy as np

B = 512
NPG = 256               # nodes per graph
N = B * NPG             # 131072
HID = 128
NCORES = 8
GPC = B // NCORES       # 64 graphs per core
NPC = N // NCORES       # 16384 nodes per core
PAD_VALUE = -10000.0

CH = 768                # nodes per full chunk (3 graphs)
NFULL = NPC // CH       # 21 full chunks
TAIL = NPC - NFULL * CH  # 256-node tail chunk (1 graph)
NCHUNK = NFULL + 1

LAST_RESULT = None      # BassKernelResults of the most recent device run
_CACHE = {}


def _numpy_ref(last_node_batch, h, W_embed, b_embed, W1, b1, W2, b2, W3, b3,
               segment_ids, max_nodes):
    """Exact host fallback (used only if inputs don't match the expected
    uniform-graph structure)."""
    lnb = np.asarray(last_node_batch, np.float32)
    h = np.asarray(h, np.float32)
    seg = np.asarray(segment_ids).astype(np.int64)
    b = lnb.shape[0]
    n = h.shape[0]
    mn = int(np.asarray(max_nodes))
    node_embed = lnb @ np.asarray(W_embed, np.float32) + np.asarray(b_embed, np.float32)
    stack = np.concatenate([h, node_embed[seg]], axis=1)
    out = np.tanh(stack @ np.asarray(W1, np.float32) + np.asarray(b1, np.float32))
    out = np.tanh(out @ np.asarray(W2, np.float32) + np.asarray(b2, np.float32))
    edges = out @ np.asarray(W3, np.float32) + np.asarray(b3, np.float32)
    counts = np.zeros(b, np.int64)
    np.add.at(counts, seg, 1)
    offsets = np.cumsum(counts) - counts
    pos = np.arange(n) - offsets[seg]
    padded = np.full((b, mn, 2), PAD_VALUE, np.float32)
    padded[seg, pos] = edges
    return padded.reshape(b, mn * 2)


def _build():
    """Build + compile the per-core Bass program (identical on all cores)."""
    import concourse.bacc as bacc
    import concourse.mybir as mybir
    import concourse.tile as tile

    f32 = mybir.dt.float32
    bf16 = mybir.dt.bfloat16
    Tanh = mybir.ActivationFunctionType.Tanh

    nc = bacc.Bacc("TRN2", target_bir_lowering=False, debug=False, enable_asserts=False)

    # wpk columns (bf16): W1h[0:256] W2a[256:512] W2b[512:768] W3a[768:770]
    #                     W3b[770:772] ones[772:1284] (ones on partition 0)
    hT = nc.dram_tensor("hT", [128, NPC], bf16, kind="ExternalInput").ap()
    wpk = nc.dram_tensor("wpk", [128, 1284], bf16, kind="ExternalInput").ap()
    # per-graph L1 contribution: ct1[0, g*256+f] = C[g, f]
    ct1 = nc.dram_tensor("ct1", [1, GPC * 256], bf16, kind="ExternalInput").ap()
    b2s = nc.dram_tensor("b2s", [128, 2], f32, kind="ExternalInput").ap()
    out_d = nc.dram_tensor("out", [2, NPC], f32, kind="ExternalOutput").ap()

    # Accumulation-group windows (node-offset, width) per (chunk-size, m).
    # Each group's PSUM output must stay inside one 2KB bank (512 fp32 cols,
    # bank boundaries at z-tile cols 512/1024), so the split differs between
    # the m0 span (cols 0:768) and the m1 span (cols 768:1536).
    def windows(ch, m):
        if ch == CH:
            return [(0, 512), (512, 256)] if m == 0 else [(0, 256), (256, 512)]
        return [(0, 256)]

    with tile.TileContext(nc) as tc:
        with (
            tc.tile_pool(name="w", bufs=1) as wp,
            tc.tile_pool(name="io", bufs=3) as io,
            tc.tile_pool(name="act", bufs=2) as ac,
            tc.tile_pool(name="ps12", bufs=1, space="PSUM") as ps12,
            tc.tile_pool(name="ps3", bufs=2, space="PSUM") as ps3,
        ):
            s_w = wp.tile([128, 1284], bf16, tag="wpk")
            nc.sync.dma_start(out=s_w[:], in_=wpk)
            s_ct = wp.tile([1, GPC * 256], bf16, tag="ct1")
            nc.sync.dma_start(out=s_ct[:], in_=ct1)
            s_b2 = wp.tile([128, 2], f32, tag="b2s")
            nc.sync.dma_start(out=s_b2[:], in_=b2s)

            s_w1 = s_w[:, 0:256]
            s_w2a = s_w[:, 256:512]
            s_w2b = s_w[:, 512:768]
            s_w3a = s_w[:, 768:770]
            s_w3b = s_w[:, 770:772]
            s_one = s_w[0:1, 772:1284]

            chunks = [(i * CH, CH) for i in range(NFULL)] + [(NFULL * CH, TAIL)]
            # column offset of the m-half within the z tiles (and y tiles)
            mbase = {CH: (0, 768), TAIL: (0, 512)}

            h_tiles = {}
            y1_t = {}
            y2_t = {}
            z2_t = {}
            p3_t = {}

            def dma_h(i):
                base, ch = chunks[i]
                t = io.tile([128, CH], bf16, tag="h")
                if i == 0:
                    nc.sync.dma_start(out=t[:, 0:ch // 2], in_=hT[:, base:base + ch // 2])
                    nc.sync.dma_start(out=t[:, ch // 2:ch],
                                      in_=hT[:, base + ch // 2:base + ch])
                else:
                    nc.sync.dma_start(out=t[:, 0:ch], in_=hT[:, base:base + ch])
                h_tiles[i] = t

            def l1(i):
                base, ch = chunks[i]
                t_h = h_tiles.pop(i)
                z1 = ps12.tile([128, 1536], f32, tag="z1")
                for m in (0, 1):
                    mb = mbase[ch][m]
                    for (o, w) in windows(ch, m):
                        nc.tensor.matmul(
                            z1[:, mb + o:mb + o + w],
                            s_w1[:, m * 128:m * 128 + 128],
                            t_h[:, o:o + w],
                            start=True, stop=False,
                        )
                        ng = w // 256
                        for j in range(ng):
                            g = (base + o) // 256 + j
                            co = g * 256 + m * 128
                            nc.tensor.matmul(
                                z1[:, mb + o + j * 256:mb + o + j * 256 + 256],
                                s_ct[:, co:co + 128],
                                s_one[:, 0:256],
                                start=False, stop=(j == ng - 1),
                            )
                # tanh over the full span (tail: middle 256 cols are unused
                # garbage, harmless)
                span = mbase[ch][1] + ch - mbase[ch][0] if ch == CH else 768
                y1 = ac.tile([128, 1536], bf16, tag="y1")
                nc.scalar.activation(y1[:, 0:span], z1[:, 0:span], Tanh)
                y1_t[i] = y1

            def l2(i):
                base, ch = chunks[i]
                y1 = y1_t.pop(i)
                mb1 = mbase[ch][1]
                z2 = ps12.tile([128, 1536], f32, tag="z2")
                for m in (0, 1):
                    mb = mbase[ch][m]
                    for (o, w) in windows(ch, m):
                        nc.tensor.matmul(
                            z2[:, mb + o:mb + o + w],
                            s_w2a[:, m * 128:m * 128 + 128],
                            y1[:, o:o + w],
                            start=True, stop=False,
                        )
                        nc.tensor.matmul(
                            z2[:, mb + o:mb + o + w],
                            s_w2b[:, m * 128:m * 128 + 128],
                            y1[:, mb1 + o:mb1 + o + w],
                            start=False, stop=True,
                        )
                y2s = ac.tile([128, 1536], bf16, tag="y2s")
                nc.vector.tensor_scalar_add(y2s[:, 0:ch], z2[:, 0:ch], s_b2[:, 0:1])
                nc.vector.tensor_scalar_add(y2s[:, mb1:mb1 + ch], z2[:, mb1:mb1 + ch],
                                            s_b2[:, 1:2])
                span = mb1 + ch
                y2 = ac.tile([128, 1536], bf16, tag="y2")
                nc.scalar.activation(y2[:, 0:span], y2s[:, 0:span], Tanh)
                y2_t[i] = y2
                z2_t[i] = z2

            def l3(i):
                base, ch = chunks[i]
                y2 = y2_t.pop(i)
                z2_t.pop(i)
                mb1 = mbase[ch][1]
                tiles = []
                for (o, w) in windows(ch, 0):
                    p = ps3.tile([2, 512], f32, tag="p3")
                    nc.tensor.matmul(p[:, 0:w], s_w3a, y2[:, o:o + w],
                                     start=True, stop=False)
                    nc.tensor.matmul(p[:, 0:w], s_w3b, y2[:, mb1 + o:mb1 + o + w],
                                     start=False, stop=True)
                    tiles.append((p, o, w))
                ed = io.tile([2, CH], f32, tag="ed")
                for (p, o, w) in tiles:
                    nc.vector.tensor_copy(ed[:, o:o + w], p[:, 0:w])
                nc.sync.dma_start(out=out_d[:, base:base + ch], in_=ed[:, 0:ch])

            dma_h(0)
            dma_h(1)
            for i in range(NCHUNK + 2):
                if i + 2 < NCHUNK:
                    dma_h(i + 2)
                if i < NCHUNK:
                    l1(i)
                if 1 <= i:
                    if i - 1 < NCHUNK:
                        l2(i - 1)
                if 2 <= i:
                    if i - 2 < NCHUNK:
                        l3(i - 2)

    nc.compile()
    return nc


def _to_bf16(x):
    import ml_dtypes
    return np.ascontiguousarray(x.astype(ml_dtypes.bfloat16))


def kernel(last_node_batch, h, W_embed, b_embed, W1, b1, W2, b2, W3, b3,
           segment_ids, max_nodes):
    global LAST_RESULT
    lnb = np.asarray(last_node_batch, np.float32)
    h = np.asarray(h, np.float32)
    seg = np.asarray(segment_ids)
    mn = int(np.asarray(max_nodes))

    expected_seg = np.repeat(np.arange(B, dtype=seg.dtype), NPG)
    if not (lnb.shape == (B, HID) and h.shape == (N, HID) and mn == NPG
            and seg.shape == (N,) and np.array_equal(seg, expected_seg)):
        return _numpy_ref(last_node_batch, h, W_embed, b_embed, W1, b1, W2, b2,
                          W3, b3, segment_ids, max_nodes)

    import sys
    try:
        import antenv.axon_hooks  # noqa: F401
    except ImportError:
        # bass_utils imports this unconditionally when tracing is requested
        # (e.g. BASS_TRACE set in the environment); provide a no-op fallback
        # so tracing degrades instead of crashing.
        import types
        _m = types.ModuleType("antenv.axon_hooks")
        _m.get_axon_ntff_profile_hook = lambda: None
        _m.set_axon_ntff_profile_hook = lambda h: None
        sys.modules["antenv.axon_hooks"] = _m

    from concourse.bass_utils import run_bass_kernel_spmd

    if "nc" not in _CACHE:
        _CACHE["nc"] = _build()
    nc = _CACHE["nc"]

    W1 = np.asarray(W1, np.float32)
    W2 = np.asarray(W2, np.float32)
    W3 = np.asarray(W3, np.float32)
    b1v = np.asarray(b1, np.float32)
    b2v = np.asarray(b2, np.float32)
    b3v = np.asarray(b3, np.float32)

    # per-graph L1 contribution: C = (lnb @ W_embed + b_embed) @ W1[H:] + b1
    ne = lnb @ np.asarray(W_embed, np.float32) + np.asarray(b_embed, np.float32)
    C = ne @ W1[HID:, :] + b1v                                  # [B, 2H]

    ones = np.zeros((128, 512), np.float32)
    ones[0, :] = 1.0
    wpk = _to_bf16(np.concatenate(
        [W1[:HID, :], W2[:HID, :], W2[HID:, :], W3[:HID, :], W3[HID:, :], ones],
        axis=1))
    b2s = np.ascontiguousarray(np.stack([b2v[:HID], b2v[HID:]], axis=1))

    in_maps = []
    for c in range(NCORES):
        m = {
            "wpk": wpk,
            "ct1": _to_bf16(C[c * GPC:(c + 1) * GPC].reshape(1, -1)),
            "b2s": b2s,
            "hT": _to_bf16(np.ascontiguousarray(h[c * NPC:(c + 1) * NPC].T)),
        }
        in_maps.append(m)

    trace = bool(int(os.environ.get("KERNEL_TRACE", "0")))
    res = run_bass_kernel_spmd(nc, in_maps, core_ids=list(range(NCORES)),
                               trace=trace)
    LAST_RESULT = res

    out = np.empty((B, NPG * 2), np.float32)
    for c in range(NCORES):
        od = res.results[c]["out"]          # [2, NPC]; [cc, n] = edges[n, cc]
        blk = od.reshape(2, GPC, NPG).transpose(1, 2, 0).reshape(GPC, NPG * 2)
        out[c * GPC:(c + 1) * GPC] = blk
    out += np.tile(b3v, NPG)[None, :]
    return out
